# revision 21
# baseline (speedup 1.0000x reference)
"""Tied-attention (MSA-style) kernel for 8 TRN2 NeuronCores.

Problem: x (32,1024,256) f32; q/kv projections; tied attention over the
r=32 MSA-row dim (logits summed over r); softmax; out-projection + bias.

Sharding: tensor-parallel by heads (8 heads -> 1 head per core). Each core
computes q/k/v for its head from the full (host-pre-transposed, bf16-cast)
x, accumulates its head's tied logits S^T = sum_r k_r q_r^T entirely
locally (no collective), softmaxes along the PSUM partition axis via a
ones-matmul, applies attention, then four pipelined 1MB AllToAll chunks
(one per 4 attention pairs) redistribute per-head outputs into per-core
row shards; each chunk's output projection interleaves with the next
chunk's attention compute so the PE stays busy during the collectives.

The tied-logit matmuls run in fp8e4 DoubleRow mode (256-deep contraction,
2 r-pairs per matmul): logits are tiny (std ~0.1 after scale) so the fp8
quantization of q/k adds only ~0.5% softmax error. v and the attention-
weighted sums stay bf16.

v is produced head-transposed and flipped to row-major with PE-mode
transposes (DMA transpose serializes the whole DMA subsystem via
xbar_mode transitions - measured 12us/pair stalls - so it is avoided).

Compute dtype: bf16/fp8 operands, f32 PSUM accumulation, f32 softmax.
"""
import numpy as np
import ml_dtypes

import concourse.bacc as bacc
import concourse.mybir as mybir
import concourse.tile as tile
from concourse.bass_utils import run_bass_kernel_spmd

dt = mybir.dt
BF16 = ml_dtypes.bfloat16

H, D, R, N, DIM = 8, 64, 32, 1024, 256
INNER = H * D          # 512
ROWS = R * N           # 32768
NPAIR = R // 2         # 16
NU = NPAIR // 2        # 8 pair-pairs for fp8 DoubleRow packing
NCORES = 8
RL = R // NCORES       # 4 rows of r per core after AllToAll
NCHUNK = 2             # A2A chunks (8 pairs -> 16 rows -> 2 rows per dest)
CPAIR = NPAIR // NCHUNK  # pairs per chunk
CROW = 2 * CPAIR // NCORES  # rows each dest receives per chunk
WSCALE = 16.0          # host pre-scales Wq/Wk so fp8 weights stay normal
ASCALE = 256.0         # av scaled into fp8e3 range for the A2A; Wout/ASCALE
SCALE = (D ** -0.5) * (R ** -0.5) / (WSCALE * WSCALE)

_NC_CACHE = None


def _build():
    nc = bacc.Bacc("TRN2", target_bir_lowering=False, debug=False, num_devices=NCORES)

    xt = nc.dram_tensor("xt", [DIM, ROWS], dt.bfloat16, kind="ExternalInput")
    x8 = nc.dram_tensor("x8", [DIM, ROWS], dt.float8e4, kind="ExternalInput")
    wq = nc.dram_tensor("wq", [DIM, D], dt.bfloat16, kind="ExternalInput")
    wk = nc.dram_tensor("wk", [DIM, D], dt.bfloat16, kind="ExternalInput")
    wv = nc.dram_tensor("wv", [DIM, D], dt.bfloat16, kind="ExternalInput")
    wout = nc.dram_tensor("wout", [INNER, DIM], dt.bfloat16, kind="ExternalInput")
    bias = nc.dram_tensor("bias", [128, 2], dt.float32, kind="ExternalInput")
    ident = nc.dram_tensor("ident", [128, 128], dt.bfloat16, kind="ExternalInput")
    yt = nc.dram_tensor("yt", [DIM, RL * N], dt.bfloat16, kind="ExternalOutput")

    with tile.TileContext(nc) as tc:
        with (
            tc.tile_pool(name="dram", bufs=1, space="DRAM") as dram,
            tc.tile_pool(name="persist", bufs=1) as per,
            tc.tile_pool(name="xc", bufs=6) as xcp,
            tc.tile_pool(name="stage", bufs=4) as stg,
            tc.tile_pool(name="gio", bufs=2) as gio,
        ):
            # One A2A chunk per 8 attention pairs (CC ops have a ~14us
            # floor here, so fewer/bigger chunks win): chunk k carries rows
            # 16k..16k+15; dest d receives rows {16k+d, 16k+8+d}.
            a2a_ins = [dram.tile([NCORES, CROW, D, N], dt.float8e3, name=f"a2ai{k}")
                       for k in range(NCHUNK)]
            a2a_outs = [dram.tile([NCORES, CROW, D, N], dt.float8e3, name=f"a2ao{k}")
                        for k in range(NCHUNK)]

            # persistent SBUF tensors
            wq_sb = per.tile([128, 2, D], dt.bfloat16, tag="wq")
            wk_sb = per.tile([128, 2, D], dt.bfloat16, tag="wk")
            wv_sb = per.tile([128, 2, D], dt.bfloat16, tag="wv")
            # q and k weights packed into one DoubleRow stationary operand:
            # out partitions 0-63 = q^T, 64-127 = k^T
            w8qk = per.tile([128, 2, 2 * D], dt.float8e4, tag="w8qk")
            wout_sb = per.tile([128, 4, DIM], dt.bfloat16, tag="wout")
            bias_sb = per.tile([128, 2], dt.float32, tag="bias")
            ident_sb = per.tile([128, 128], dt.bfloat16, tag="ident")
            ones_col = per.tile([128, 1], dt.bfloat16, tag="ones_col")
            den_sb = per.tile([1, N], dt.float32, tag="den")
            bcf_sb = per.tile([128, N], dt.float32, tag="bcf")
            # per-pair-pair persistent q^T/k^T in fp8e4 for DoubleRow:
            # [(r-parity, d), pair-in-u, n]; v row-major bf16 per pair
            qts = [per.tile([128, 2, N], dt.float8e4, tag=f"qt{u}", name=f"qt{u}")
                   for u in range(NU)]
            kts = [per.tile([128, 2, N], dt.float8e4, tag=f"kt{u}", name=f"kt{u}")
                   for u in range(NU)]
            vs = [per.tile([128, 8, 128], dt.bfloat16, tag=f"v{p}", name=f"v{p}")
                  for p in range(NPAIR)]
            # P^T tiles per jc: [j-in-chunk, i]
            pts = [per.tile([128, N], dt.bfloat16, tag=f"pt{jc}", name=f"pt{jc}")
                   for jc in range(8)]

            nc.gpsimd.dma_start(wq_sb[:], wq.ap().rearrange("(a p) m -> p a m", p=128))
            nc.gpsimd.dma_start(wk_sb[:], wk.ap().rearrange("(a p) m -> p a m", p=128))
            nc.gpsimd.dma_start(wv_sb[:], wv.ap().rearrange("(a p) m -> p a m", p=128))
            nc.gpsimd.dma_start(ident_sb[:], ident[:])
            nc.vector.memset(ones_col[:], 1.0)
            nc.scalar.activation(w8qk[:, :, 0:D], wq_sb[:],
                                 mybir.ActivationFunctionType.Copy)
            nc.scalar.activation(w8qk[:, :, D:2 * D], wk_sb[:],
                                 mybir.ActivationFunctionType.Copy)

            # warm-up collective: absorbs cross-core start skew and ncfw cold
            # init while phase 1 computes; CC engine only.
            warm_in = dram.tile([1, 64], dt.float32, name="warm_in")
            warm_out = dram.tile([NCORES, 64], dt.float32, name="warm_out")
            nc.gpsimd.collective_compute(
                "AllGather",
                mybir.AluOpType.bypass,
                replica_groups=[list(range(NCORES))],
                ins=[warm_in.opt()],
                outs=[warm_out.opt()],
            )

            # ---- Phase 1: projections q^T,k^T (parity layout, fp8) + v ----
            with (
                tc.tile_pool(name="ps_proj", bufs=3, space="PSUM") as psp,
                tc.tile_pool(name="ps_vtr", bufs=2, space="PSUM") as psv,
            ):
                for p in range(NPAIR):
                    u, pu = p // 2, p % 2
                    # x loads spread across all three DMA trigger queues so no
                    # single SWDGE queue (~170 GB/s) floors the phase
                    xc = [xcp.tile([128, 2 * N], dt.bfloat16, tag="xc", name=f"xc{p}_{i}")
                          for i in range(2)]
                    nc.sync.dma_start(
                        xc[0][:], xt[0:128, 2 * p * N:(2 * p + 2) * N])
                    nc.scalar.dma_start(
                        xc[1][:], xt[128:256, 2 * p * N:(2 * p + 2) * N])
                    # fp8 x for the q/k DoubleRow projections, host-cast:
                    # [c-part, kt, (parity, n)]
                    xc8 = xcp.tile([128, 2, 2 * N], dt.float8e4, tag="xc8",
                                   name=f"xc8{p}")
                    for kt in range(2):
                        nc.gpsimd.dma_start(
                            xc8[:, kt, :],
                            x8[kt * 128:(kt + 1) * 128,
                               2 * p * N:(2 * p + 2) * N])
                    # q/k: one fp8 DoubleRow matmul per (parity, n-half) does
                    # the full 256-deep contraction for q AND k at once
                    pqk = [psp.tile([128, N], dt.float32, tag="proj",
                                    name=f"pqk{p}_{par}") for par in range(2)]
                    for par in range(2):
                        for nh in range(2):
                            base = par * N + nh * 512
                            nc.tensor.matmul(
                                pqk[par][:, nh * 512:nh * 512 + 512],
                                w8qk[:], xc8[:, :, base:base + 512],
                                start=True, stop=True,
                                perf_mode=mybir.MatmulPerfMode.DoubleRow)
                    pv = psp.tile([128, N], dt.float32, tag="proj", name=f"pv{p}")
                    for kt in range(2):
                        for nh in range(2):
                            sl0 = slice(nh * 512, nh * 512 + 512)
                            sl1 = slice(N + nh * 512, N + nh * 512 + 512)
                            nc.tensor.matmul(
                                pv[0:64, nh * 512:nh * 512 + 512],
                                wv_sb[:, kt, :], xc[kt][:, sl0],
                                start=(kt == 0), stop=(kt == 1),
                                tile_position=(0, 0))
                            nc.tensor.matmul(
                                pv[64:128, nh * 512:nh * 512 + 512],
                                wv_sb[:, kt, :], xc[kt][:, sl1],
                                start=(kt == 0), stop=(kt == 1),
                                tile_position=(0, 64))
                    # evacuate into parity-packed fp8 tiles (partition-shifted
                    # copies); q halves on ScalarE, k halves on VectorE
                    nc.scalar.activation(qts[u][0:64, pu, :], pqk[0][0:64, :],
                                         mybir.ActivationFunctionType.Copy)
                    nc.scalar.activation(qts[u][64:128, pu, :], pqk[1][0:64, :],
                                         mybir.ActivationFunctionType.Copy)
                    nc.vector.tensor_copy(kts[u][0:64, pu, :], pqk[0][64:128, :])
                    nc.vector.tensor_copy(kts[u][64:128, pu, :], pqk[1][64:128, :])
                    vstage = stg.tile([128, N], dt.bfloat16, tag="vstage",
                                      name=f"vst{p}")
                    nc.vector.tensor_copy(vstage[:], pv[:])
                    # PE-transpose v^T (parity,d)xn -> n x (parity,d); all 8
                    # transposes land in one psum bank, one evac copy
                    pt_ps = psv.tile([128, 8, 128], dt.bfloat16, tag="vtr",
                                     name=f"vtr{p}")
                    for jc in range(8):
                        nc.tensor.transpose(pt_ps[:, jc, :],
                                            vstage[:, jc * 128:(jc + 1) * 128],
                                            ident_sb[:])
                    nc.vector.tensor_copy(vs[p][:], pt_ps[:])

            # tail-phase weights, loaded once phase 1's queue traffic drains
            nc.gpsimd.dma_start(wout_sb[:], wout.ap().rearrange("(a p) m -> p a m", p=128))
            nc.gpsimd.dma_start(bias_sb[:], bias[:])

            # ---- Phase 2: S^T = sum_r k_r q_r^T (fp8 DoubleRow), softmax ----
            with (
                tc.tile_pool(name="ps_s", bufs=3, space="PSUM") as pss,
                tc.tile_pool(name="ps_den", bufs=1, space="PSUM") as psd,
            ):
                pden = psd.tile([1, N], dt.float32, tag="den")
                for jc in range(8):
                    ps = pss.tile([128, N], dt.float32, tag="s", name=f"s{jc}")
                    for u in range(NU):
                        for ih in range(2):
                            nc.tensor.matmul(
                                ps[:, ih * 512:ih * 512 + 512],
                                kts[u][:, :, jc * 128:(jc + 1) * 128],
                                qts[u][:, :, ih * 512:ih * 512 + 512],
                                start=(u == 0), stop=(u == NU - 1),
                                perf_mode=mybir.MatmulPerfMode.DoubleRow)
                    nc.scalar.activation(pts[jc][:], ps[:],
                                         mybir.ActivationFunctionType.Exp,
                                         scale=SCALE)
                    for ih in range(2):
                        nc.tensor.matmul(pden[:, ih * 512:ih * 512 + 512],
                                         ones_col[:],
                                         pts[jc][:, ih * 512:ih * 512 + 512],
                                         start=(jc == 0), stop=(jc == 7))
                # 1/ASCALE fold: bcf = ASCALE/den scales av into fp8e3 range
                nc.scalar.activation(den_sb[:], pden[:],
                                     mybir.ActivationFunctionType.Copy,
                                     scale=1.0 / ASCALE)
            # broadcast first, then full-width reciprocal (fast on 128 lanes)
            nc.gpsimd.partition_broadcast(bcf_sb[:], den_sb[:])
            nc.vector.reciprocal(bcf_sb[:], bcf_sb[:])

            # ---- Phase 3/4/5 pipeline: per chunk k (pairs 4k..4k+3) compute
            # attention-weighted values, fire its A2A, and run the previous
            # chunk's output projection under the collective.
            warm2_in = dram.tile([1, 64], dt.float32, name="warm2_in")
            warm2_out = dram.tile([NCORES, 64], dt.float32, name="warm2_out")
            nc.gpsimd.collective_compute(
                "AllGather",
                mybir.AluOpType.bypass,
                replica_groups=[list(range(NCORES))],
                ins=[warm2_in.opt()],
                outs=[warm2_out.opt()],
            )
            with (
                tc.tile_pool(name="ps_av", bufs=2, space="PSUM") as psa,
                tc.tile_pool(name="ps_y", bufs=2, space="PSUM") as psy,
            ):
                def yproj(k, cr):
                    # output projection for own row 16k + 8*cr + c
                    rg = 2 * k + cr  # row-group index in yt
                    g = gio.tile([128, 4, N], dt.float8e3, tag="g",
                                 name=f"g{k}_{cr}")
                    for kt in range(4):
                        nc.scalar.dma_start(g[0:64, kt, :],
                                            a2a_outs[k][2 * kt, cr, :, :])
                        nc.scalar.dma_start(g[64:128, kt, :],
                                            a2a_outs[k][2 * kt + 1, cr, :, :])
                    for m in range(2):
                        sl_m = slice(m * 128, m * 128 + 128)
                        py = psy.tile([128, N], dt.float32, tag="y",
                                      name=f"py{k}_{cr}_{m}")
                        for kt in range(4):
                            for nh in range(2):
                                nc.tensor.matmul(py[:, nh * 512:nh * 512 + 512],
                                                 wout_sb[:, kt, sl_m],
                                                 g[:, kt, nh * 512:nh * 512 + 512],
                                                 start=(kt == 0), stop=(kt == 3))
                        ysb = gio.tile([128, N], dt.bfloat16, tag="ysb",
                                       name=f"ysb{k}_{cr}_{m}")
                        if m == 0:
                            nc.vector.tensor_scalar_add(ysb[:], py[:],
                                                        bias_sb[:, m:m + 1])
                        else:
                            nc.scalar.activation(ysb[:], py[:],
                                                 mybir.ActivationFunctionType.Identity,
                                                 bias=bias_sb[:, m:m + 1])
                        # y stores on the sync queue: the gpsimd queue holds the
                        # collective triggers, which must not wait behind them
                        nc.sync.dma_start(yt[sl_m, rg * N:(rg + 1) * N], ysb[:])

                for k in range(NCHUNK):
                    for p in range(CPAIR * k, CPAIR * (k + 1)):
                        po = psa.tile([128, N], dt.float32, tag="av", name=f"av{p}")
                        for jc in range(8):
                            for ih in range(2):
                                nc.tensor.matmul(
                                    po[:, ih * 512:ih * 512 + 512],
                                    vs[p][:, jc, :],
                                    pts[jc][:, ih * 512:ih * 512 + 512],
                                    start=(jc == 0), stop=(jc == 7))
                        osb = stg.tile([128, N], dt.float8e3, tag="osb",
                                       name=f"osb{p}")
                        # normalize by the softmax denominator on evacuation
                        nc.vector.tensor_mul(osb[:], po[:], bcf_sb[:])
                        for hr in range(2):
                            r = 2 * p + hr - 16 * k
                            nc.sync.dma_start(a2a_ins[k][r % 8, r // 8, :, :],
                                              osb[64 * hr:64 * hr + 64, :])
                    nc.gpsimd.collective_compute(
                        "AllToAll",
                        mybir.AluOpType.bypass,
                        replica_groups=[list(range(NCORES))],
                        ins=[a2a_ins[k].opt()],
                        outs=[a2a_outs[k].opt()],
                    )
                    if k >= 1:
                        for cr in range(CROW):
                            yproj(k - 1, cr)
                for cr in range(CROW):
                    yproj(NCHUNK - 1, cr)
    nc.finalize()
    return nc


def make_in_maps(x, Wq, Wkv, Wout, bout):
    x = np.asarray(x, dtype=np.float32)
    xtf = np.ascontiguousarray(x.reshape(ROWS, DIM).T)
    xt = xtf.astype(BF16)
    xt8 = xtf.astype(ml_dtypes.float8_e4m3)
    wout_b = (np.asarray(Wout, np.float32) / ASCALE).astype(BF16)
    bias_b = np.ascontiguousarray(
        np.asarray(bout, np.float32).reshape(2, 128).T).astype(np.float32)
    ident = np.eye(128, dtype=BF16)
    Wq = np.asarray(Wq, np.float32)
    Wkv = np.asarray(Wkv, np.float32)

    in_maps = []
    for c in range(NCORES):
        sl = slice(c * D, (c + 1) * D)
        in_maps.append({
            "xt": xt,
            "x8": xt8,
            "wq": np.ascontiguousarray(Wq[:, sl] * WSCALE).astype(BF16),
            "wk": np.ascontiguousarray(Wkv[:, sl] * WSCALE).astype(BF16),
            "wv": np.ascontiguousarray(Wkv[:, INNER + c * D:INNER + (c + 1) * D]).astype(BF16),
            "wout": wout_b,
            "bias": bias_b,
            "ident": ident,
        })
    return in_maps


def kernel(x, Wq, Wkv, Wout, bout, tie_attn_dim):
    global _NC_CACHE
    assert int(tie_attn_dim) == R
    in_maps = make_in_maps(x, Wq, Wkv, Wout, bout)

    if _NC_CACHE is None:
        _NC_CACHE = _build()
    last_err = None
    for _attempt in range(3):
        try:
            res = run_bass_kernel_spmd(_NC_CACHE, in_maps,
                                       core_ids=list(range(NCORES)))
            break
        except Exception as e:  # transient NRT device errors; retry
            last_err = e
    else:
        raise last_err

    y = np.empty((R, N, DIM), dtype=np.float32)
    for c in range(NCORES):
        ytc = np.asarray(res.results[c]["yt"], np.float32).reshape(DIM, RL, N)  # row-group rg = r//8
        for rg in range(RL):
            y[c + 8 * rg] = ytc[:, rg, :].T
    return y
